# revision 1
# baseline (speedup 1.0000x reference)
"""Trainium2 Bass kernel for bare-Coulomb GNN message passing.

potential[t] = 0.5 * sum_{pairs} 1/r * charges[s]  (both directions), computed as:
  - host: index-only layout — directed contributions (t <- s) are atom-sharded
    across 8 cores (12500 atoms/core), assigned to the SBUF partition owning t,
    sectioned by source bank (int16 indexing for the MoE dma_gather), t-sorted.
  - device per core: Ant dma_gather pulls charges rows (padded to 256 B) per
    contribution; DVE computes 0.5/r * q and a per-partition chained prefix
    scan per channel; prefix is streamed to DRAM; per-atom run-end prefix
    values are gathered back (per-partition indirect DMA) and differenced to
    yield per-atom sums. Partition-prefix offsets cancel in the differences,
    so no cross-partition correction or collective is needed.
  - host: concatenate the 8 per-core outputs.
"""
import numpy as np

N_ATOMS = 100000
N_CHANNELS = 4
NCORES = 8
AT_CORE = N_ATOMS // NCORES          # 12500
P = 128
R_AT = 98                            # atoms per partition row-grid (98*128=12544)
GRID = P * R_AT                      # 12544
BANKS = 4
BANK_ROWS = 25600                    # 4*25600 = 102400 >= N_ATOMS
CALL_NI = 1024                       # indices per dma_gather call (HW ring cap)
GCALLS = 8                           # gather calls per super-chunk
SPC_G = CALL_NI // P                 # 8 slots per partition per gather call
SPC = SPC_G * GCALLS                 # 64 slots per partition per super-chunk
PAD_DIST = 1.0e30                    # pad slots get w ~ 1e-30 ~ 0

_CACHE = {}


def _preprocess(neighbor_indices, neighbor_distances):
    """Host-side index-metadata layout. Returns per-core input arrays + consts."""
    idx = np.asarray(neighbor_indices).astype(np.int64)
    dist = np.asarray(neighbor_distances).astype(np.float32)
    t = np.concatenate([idx[:, 0], idx[:, 1]])
    s = np.concatenate([idx[:, 1], idx[:, 0]])
    dd = np.concatenate([dist, dist])

    core = t // AT_CORE
    tl = t - core * AT_CORE                       # local atom id
    p = tl // R_AT                                # owning partition
    r = tl - p * R_AT
    b = s // BANK_ROWS                            # source bank
    s16 = (s - b * BANK_ROWS).astype(np.int16)

    # per (core, p, b) counts -> global section sizes
    gidx = (core * P + p) * BANKS + b
    cnt_pb = np.bincount(gidx, minlength=NCORES * P * BANKS).reshape(NCORES, P, BANKS)
    S_list = []
    for bb in range(BANKS):
        m = int(cnt_pb[:, :, bb].max())
        S_list.append(((m + SPC - 1) // SPC) * SPC)
    S = int(sum(S_list))
    base_b = np.concatenate([[0], np.cumsum(S_list)]).astype(np.int64)  # [5]
    ncalls = S // SPC
    bank_of_call = np.repeat(np.arange(BANKS), np.array(S_list) // SPC)

    # order contributions by (core, p, b, t); position within group
    order = np.lexsort((tl, b, p, core))
    co, po, bo, s16o, ddo, tlo, ro = (core[order], p[order], b[order],
                                      s16[order], dd[order], tl[order], r[order])
    g = (co * P + po) * BANKS + bo
    # start offset of each group in the sorted stream
    grp_counts = np.bincount(g, minlength=NCORES * P * BANKS)
    grp_starts = np.concatenate([[0], np.cumsum(grp_counts)[:-1]])
    within = np.arange(len(g)) - grp_starts[g]
    slot = base_b[bo] + within                    # slot within partition stream

    # per (core, p, b, r) cumulative end counts -> E positions
    g3 = ((co * P + po) * BANKS + bo) * R_AT + ro
    cnt3 = np.bincount(g3, minlength=NCORES * P * BANKS * R_AT) \
             .reshape(NCORES, P, BANKS, R_AT)
    endcnt = np.cumsum(cnt3, axis=3)              # inclusive
    # Eoff: per-bank prefix tensors, rows p*(S_b+1) + endcnt (+1 zero/carry col)
    pp = np.arange(P).reshape(1, P, 1, 1)
    sb1 = np.array([sl + 1 for sl in S_list]).reshape(1, 1, BANKS, 1)
    eoff = pp * sb1 + endcnt
    eoff = eoff.astype(np.int32)                  # [NCORES, P, BANKS, R_AT]

    per_core = []
    for k in range(NCORES):
        m = co == k
        pk, sk, dk, slk = po[m], s16o[m], ddo[m], slot[m]
        src = np.zeros((P, S), dtype=np.int16)
        dts = np.full((P, S), PAD_DIST, dtype=np.float32)
        src[pk, slk] = sk
        dts[pk, slk] = dk
        # wrapped idx tiles: 1024-idx gather calls, 8 per super-chunk.
        ng = ncalls * GCALLS  # number of 1024-idx gather calls
        src_calls = src.reshape(P, ng, SPC_G).transpose(1, 0, 2)   # [ng, P, 8]
        lists = src_calls.transpose(0, 2, 1).reshape(ng, CALL_NI)  # k2 = p + 128*u
        w16 = lists.reshape(ng, CALL_NI // 16, 16).transpose(0, 2, 1)  # [ng,16,64]
        wr = np.tile(w16, (1, 8, 1))                                # [ng, 128, 64]
        wrapped = wr.reshape(ncalls, GCALLS, P, CALL_NI // 16) \
                    .transpose(0, 2, 1, 3).reshape(ncalls * P, GCALLS * (CALL_NI // 16))
        eo = eoff[k].transpose(0, 2, 1).reshape(P, BANKS * R_AT)  # j = b*R_AT + r? see below
        # we want Eoffs[p, j] with j = b*R_AT + r:
        eo = eoff[k].reshape(P, BANKS * R_AT)
        per_core.append({
            "idx": wrapped,
            "dist": dts,
            "eoff": np.ascontiguousarray(eo),
        })
    consts = {"S": S, "S_list": S_list, "ncalls": ncalls,
              "bank_of_call": bank_of_call.tolist()}
    return per_core, consts


def _pad_table(charges):
    tab = np.zeros((BANKS * BANK_ROWS, 64), dtype=np.float32)
    tab[:N_ATOMS, :N_CHANNELS] = np.asarray(charges, dtype=np.float32)
    return tab


def _build_bass(S, ncalls, bank_of_call, S_list):
    import os
    import concourse.bacc as bacc
    import concourse.tile as tile
    import concourse.bass as bass
    from concourse import mybir
    bisect = os.environ.get("KBISECT", "full")

    NJ = BANKS * R_AT  # 392 boundary values per partition

    nc = bacc.Bacc("TRN2", target_bir_lowering=False, debug=False,
                   num_devices=NCORES, num_swdge_queues=2)
    table = nc.dram_tensor("table", [BANKS * BANK_ROWS, 64], mybir.dt.float32,
                           kind="ExternalInput").ap()
    idx_d = nc.dram_tensor("idx", [ncalls * P, GCALLS * (CALL_NI // 16)], mybir.dt.int16,
                           kind="ExternalInput").ap()
    dist_d = nc.dram_tensor("dist", [P, S], mybir.dt.float32,
                            kind="ExternalInput").ap()
    eoff_d = nc.dram_tensor("eoff", [P, NJ], mybir.dt.int32,
                            kind="ExternalInput").ap()
    prefixes = [nc.dram_tensor(f"prefix{b}", [P * (S_list[b] + 1), N_CHANNELS],
                               mybir.dt.float32, kind="Internal").ap()
                for b in range(BANKS)]
    out_d = nc.dram_tensor("out", [GRID, N_CHANNELS], mybir.dt.float32,
                           kind="ExternalOutput").ap()

    prefix3s = [pr.rearrange("(p s) c -> p s c", p=P) for pr in prefixes]
    call_base = []
    acc = 0
    for b in range(BANKS):
        call_base.append(acc)
        acc += S_list[b] // SPC
    out3 = out_d.rearrange("(p r) c -> p r c", p=P)

    with tile.TileContext(nc) as tc:
        with tc.tile_pool(name="idxp", bufs=6) as idxp, \
             tc.tile_pool(name="gp", bufs=6) as gp, \
             tc.tile_pool(name="wp", bufs=4) as wp, \
             tc.tile_pool(name="pfp", bufs=4) as pfp, \
             tc.tile_pool(name="persist", bufs=1) as pers:

            zt = pers.tile([P, 1, N_CHANNELS], mybir.dt.float32)
            nc.gpsimd.memset(zt[:], 0.0)
            nc.sync.dma_start(prefix3s[0][:, 0:1, :], zt[:])

            carry = pers.tile([P, N_CHANNELS], mybir.dt.float32)
            nc.gpsimd.memset(carry[:], 0.0)

            eoffs = pers.tile([P, NJ], mybir.dt.int32)
            nc.sync.dma_start(eoffs[:], eoff_d[:])

            for c in range(ncalls):
                bk = bank_of_call[c]
                it = idxp.tile([P, GCALLS * (CALL_NI // 16)], mybir.dt.int16, tag="it")
                nc.sync.dma_start(it[:], idx_d[c * P:(c + 1) * P, :])
                g = gp.tile([P, SPC, 64], mybir.dt.float32, tag="g")
                if "nogather" in bisect:
                    nc.gpsimd.memset(g[:], 1.0)
                else:
                    W16 = CALL_NI // 16
                    for ci in range(GCALLS):
                        nc.gpsimd.dma_gather(
                            out_ap=g[:, ci * SPC_G:(ci + 1) * SPC_G, :],
                            in_ap=table[bk * BANK_ROWS:(bk + 1) * BANK_ROWS, :],
                            idxs_ap=it[:, ci * W16:(ci + 1) * W16],
                            num_idxs=CALL_NI, num_idxs_reg=CALL_NI, elem_size=64,
                            single_packet=False, queue_num=ci % 2,
                        )
                dt_ = wp.tile([P, SPC], mybir.dt.float32, tag="dt")
                nc.sync.dma_start(dt_[:], dist_d[:, c * SPC:(c + 1) * SPC])
                wt = wp.tile([P, SPC], mybir.dt.float32, tag="wt")
                nc.vector.reciprocal(wt[:], dt_[:])
                v = wp.tile([P, SPC, N_CHANNELS], mybir.dt.float32, tag="v")
                nc.vector.scalar_tensor_tensor(
                    out=v[:], in0=g[:, :, 0:N_CHANNELS], scalar=0.5,
                    in1=wt[:, :, None].to_broadcast([P, SPC, N_CHANNELS]),
                    op0=mybir.AluOpType.mult, op1=mybir.AluOpType.mult,
                )
                pf = pfp.tile([P, SPC, N_CHANNELS], mybir.dt.float32, tag="pf")
                for ch in range(N_CHANNELS):
                    nc.vector.tensor_tensor_scan(
                        out=pf[:, :, ch], data0=v[:, :, ch], data1=v[:, :, ch],
                        initial=carry[:, ch:ch + 1],
                        op0=mybir.AluOpType.add, op1=mybir.AluOpType.bypass,
                    )
                nc.vector.tensor_copy(carry[:], pf[:, SPC - 1, :])
                cl = c - call_base[bk]
                nc.sync.dma_start(
                    prefix3s[bk][:, 1 + cl * SPC:1 + (cl + 1) * SPC, :], pf[:])
                if bk + 1 < BANKS and c + 1 == call_base[bk + 1]:
                    nc.sync.dma_start(prefix3s[bk + 1][:, 0:1, :],
                                      pf[:, SPC - 1:SPC, :])

            et = pers.tile([P, NJ * N_CHANNELS], mybir.dt.float32)
            if "noj" in bisect:
                nc.gpsimd.memset(et[:], 0.0)
            else:
                for j in range(NJ):
                    nc.gpsimd.indirect_dma_start(
                        out=et[:, 4 * j:4 * (j + 1)],
                        out_offset=None,
                        in_=prefixes[j // R_AT][:],
                        in_offset=bass.IndirectOffsetOnAxis(
                            ap=eoffs[:, j:j + 1], axis=0),
                    )
            dt2 = pers.tile([P, NJ * N_CHANNELS], mybir.dt.float32)
            nc.vector.tensor_copy(dt2[:, 0:4], et[:, 0:4])
            nc.vector.tensor_tensor(
                out=dt2[:, 4:], in0=et[:, 4:], in1=et[:, 0:NJ * 4 - 4],
                op=mybir.AluOpType.subtract)
            SEC = R_AT * N_CHANNELS
            o1 = pers.tile([P, SEC], mybir.dt.float32)
            o2 = pers.tile([P, SEC], mybir.dt.float32)
            nc.vector.tensor_add(o1[:], dt2[:, 0:SEC], dt2[:, SEC:2 * SEC])
            nc.vector.tensor_add(o2[:], dt2[:, 2 * SEC:3 * SEC], dt2[:, 3 * SEC:4 * SEC])
            ot = pers.tile([P, R_AT, N_CHANNELS], mybir.dt.float32)
            o1v = o1[:].rearrange("p (r c) -> p r c", c=N_CHANNELS)
            o2v = o2[:].rearrange("p (r c) -> p r c", c=N_CHANNELS)
            nc.vector.tensor_add(ot[:], o1v, o2v)
            nc.sync.dma_start(out3[:, :, :], ot[:])
    nc.finalize()
    return nc


class _Runner:
    def __init__(self, nc, n_cores):
        import jax
        from jax.sharding import Mesh, PartitionSpec
        try:
            from jax.experimental.shard_map import shard_map
        except Exception:
            from jax.sharding import shard_map
        from concourse import mybir
        from concourse.bass2jax import (_bass_exec_p, partition_id_tensor,
                                        install_neuronx_cc_hook)
        install_neuronx_cc_hook()
        self.jax = jax
        self.n_cores = n_cores
        pname = nc.partition_id_tensor.name if nc.partition_id_tensor else None
        in_names, out_names, out_avals, zero_outs = [], [], [], []
        for alloc in nc.m.functions[0].allocations:
            if not isinstance(alloc, mybir.MemoryLocationSet):
                continue
            name = alloc.memorylocations[0].name
            if alloc.kind == "ExternalInput":
                if name != pname:
                    in_names.append(name)
            elif alloc.kind == "ExternalOutput":
                shape = tuple(alloc.tensor_shape)
                dtype = mybir.dt.np(alloc.dtype)
                out_names.append(name)
                out_avals.append(jax.core.ShapedArray(shape, dtype))
                zero_outs.append(np.zeros(shape, dtype))
        self.in_names, self.out_names = in_names, out_names
        self.out_avals, self.zero_outs = out_avals, zero_outs
        n_params, n_outs = len(in_names), len(out_names)
        all_in = list(in_names) + list(out_names)
        if pname is not None:
            all_in.append(pname)

        def _body(*args):
            operands = list(args)
            if pname is not None:
                operands.append(partition_id_tensor())
            outs = _bass_exec_p.bind(
                *operands, out_avals=tuple(out_avals), in_names=tuple(all_in),
                out_names=tuple(out_names), lowering_input_output_aliases=(),
                sim_require_finite=False, sim_require_nnan=False, nc=nc)
            return tuple(outs)

        devices = jax.devices()[:n_cores]
        mesh = Mesh(np.asarray(devices), ("core",))
        in_specs = (PartitionSpec("core"),) * (n_params + n_outs)
        out_specs = (PartitionSpec("core"),) * n_outs
        self.fn = jax.jit(
            shard_map(_body, mesh=mesh, in_specs=in_specs,
                      out_specs=out_specs, check_rep=False),
            keep_unused=True)

    def run(self, in_maps):
        jax = self.jax
        concat_in = [
            np.concatenate([np.asarray(in_maps[c][n]) for c in range(self.n_cores)], axis=0)
            for n in self.in_names]
        concat_zeros = [
            np.zeros((self.n_cores * z.shape[0], *z.shape[1:]), z.dtype)
            for z in self.zero_outs]
        dargs = [jax.device_put(a) for a in concat_in + concat_zeros]
        outs = self.fn(*dargs)
        jax.block_until_ready(outs)
        res = []
        for c in range(self.n_cores):
            d = {}
            for i, n in enumerate(self.out_names):
                a = np.asarray(outs[i]).reshape(self.n_cores, *self.out_avals[i].shape)
                d[n] = a[c]
            res.append(d)
        return res


def kernel(charges, cell, positions, neighbor_indices, neighbor_distances):
    per_core, consts = _preprocess(neighbor_indices, neighbor_distances)
    key = (consts["S"], consts["ncalls"], tuple(consts["bank_of_call"]))
    if key not in _CACHE:
        nc = _build_bass(consts["S"], consts["ncalls"], consts["bank_of_call"],
                         consts["S_list"])
        _CACHE[key] = _Runner(nc, NCORES)
    runner = _CACHE[key]
    tab = _pad_table(charges)
    in_maps = [{"table": tab, "idx": pc["idx"], "dist": pc["dist"],
                "eoff": pc["eoff"]} for pc in per_core]
    res = runner.run(in_maps)
    out = np.concatenate([res[k]["out"][:AT_CORE] for k in range(NCORES)], axis=0)
    return out.astype(np.float32)



# revision 15
# speedup vs baseline: 34.5580x; 34.5580x over previous
"""Trainium2 Bass kernel for bare-Coulomb GNN message passing.

potential[t] = 0.5 * sum_{pairs} (1/r) * charges[s]  (both directions).

Design (per NeuronCore; 8 cores shard targets):
  - charges table lives in SBUF, fp32, laid out as 16 distinct rows
    (source-bank b in 0..3  x  channel c in 0..3) replicated across the
    8 GPSIMD 16-partition groups: row 16g+4b+c = 0.5-foldable channel c
    of source bank b (25000 atoms).
  - directed edges (6.4M total) are sharded by target: core = t//12500,
    GPSIMD group g = local_t//1568. Each group has ONE edge stream
    (shared by its 16 partitions), sectioned by (source bank b, atom
    segment seg of 196 targets), target-sorted within a section.
  - per section (one ap_gather call each, alternating between two ring
    tiles): Act-engine reciprocal of bf16 distances -> w (fp32, written
    into the pf ring), gpsimd ap_gather pulls q[s] for all 16 (b,c)
    rows at the shared per-group index, DVE multiplies (0.5*g)*w and
    running-scans the result in place (prefix per partition row).
  - per section: gpsimd ap_gather extracts per-atom run-end prefix
    values (+ carry column at ring slot start); adjacent differences
    give per-(b,c,atom) partials; garbage from foreign-bank rows
    cancels in the differences.
  - per bank: DVE stream_shuffle aligns rows 16g+4b+c onto 16g+c and
    accumulates over b. Output rows 16g+c DMA to DRAM.
"""
import numpy as np

N_ATOMS = 100000
C = 4
NCORES = 8
G = 8                 # gpsimd groups (16 partitions each)
P = 128
ATC = 12500           # atoms per core
ATG = 1568            # atom slots per group (8*1568 = 12544 >= 12500)
SEGS = 8              # atom segments per bank-section
SEGA = ATG // SEGS    # 196
BANKS = 4
BANKA = N_ATOMS // BANKS  # 25000
NSEC = BANKS * SEGS   # 32 sections, order (b major, seg minor)
EB = 224              # entries per section block: 1 dummy + 196 + 27 pad
                      # (multiple of 32 so per-section idx slices stay
                      #  4-byte aligned — the gather ucode reads the int16
                      #  index stream as 32-bit vectors)
PAD_DIST = 2.0

_CACHE = {}


def _roundup(x, m):
    return (x + m - 1) // m * m


def _preprocess(neighbor_indices, neighbor_distances):
    idx = np.asarray(neighbor_indices).astype(np.int64)
    dist = np.asarray(neighbor_distances).astype(np.float32)
    t = np.concatenate([idx[:, 0], idx[:, 1]])
    s = np.concatenate([idx[:, 1], idx[:, 0]])
    dd = np.concatenate([dist, dist])

    core = t // ATC
    tl = t - core * ATC
    g = tl // ATG
    a = tl - g * ATG                 # 0..1567
    b = s // BANKA
    si = (s - b * BANKA).astype(np.int16)
    seg = a // SEGA
    aa = a - seg * SEGA              # 0..195
    sec = b * SEGS + seg             # 0..31

    lin = (core * G + g) * NSEC + sec
    cnt_sec = np.bincount(lin, minlength=64 * NSEC).reshape(64, NSEC)
    Lsec = np.array([_roundup(int(m), 16) for m in cnt_sec.max(axis=0)])
    sec_off = np.concatenate([[0], np.cumsum(Lsec)]).astype(np.int64)
    S = int(sec_off[-1])

    order = np.lexsort((a, sec, g, core))
    co, go, seco = core[order], g[order], sec[order]
    sio, ddo = si[order], dd[order]
    aao = aa[order]

    lin_o = (co * G + go) * NSEC + seco
    grp_cnt = np.bincount(lin_o, minlength=64 * NSEC)
    starts = np.concatenate([[0], np.cumsum(grp_cnt)[:-1]])
    within = np.arange(len(lin_o)) - starts[lin_o]
    pos = sec_off[seco] + within

    idx_stream = np.zeros((NCORES, G, S), np.int16)
    dist_stream = np.full((NCORES, G, S), PAD_DIST, np.float32)
    idx_stream[co, go, pos] = sio
    dist_stream[co, go, pos] = ddo

    # per-atom inclusive end counts within each section
    lin4 = lin_o * SEGA + aao
    cnt4 = np.bincount(lin4, minlength=64 * NSEC * SEGA) \
             .reshape(64, NSEC, SEGA)
    ends = np.cumsum(cnt4, axis=2).astype(np.int16)  # <= Lsec <= 3456

    eidx = np.zeros((64, NSEC, EB), np.int16)
    eidx[:, :, 1:1 + SEGA] = ends   # ring pos = ends (0 -> carry col)

    import ml_dtypes
    per_core = []
    for k in range(NCORES):
        base = k * G
        iw = idx_stream[k].reshape(G, S // 16, 16).transpose(0, 2, 1) \
                          .reshape(P, S // 16)
        ew = eidx[base:base + G].reshape(G, NSEC * EB // 16, 16) \
                                .transpose(0, 2, 1).reshape(P, NSEC * EB // 16)
        dr = np.repeat(dist_stream[k].astype(ml_dtypes.bfloat16), 16, axis=0)
        per_core.append({"idx": np.ascontiguousarray(iw),
                         "eidx": np.ascontiguousarray(ew),
                         "dist": np.ascontiguousarray(dr)})
    consts = {"Lsec": tuple(int(x) for x in Lsec), "S": S}
    return per_core, consts


def _build_table(charges):
    ch = np.asarray(charges, dtype=np.float32)
    tab16 = np.zeros((16, BANKA), np.float32)
    for b in range(BANKS):
        for c in range(C):
            tab16[4 * b + c] = ch[b * BANKA:(b + 1) * BANKA, c]
    return np.tile(tab16, (G, 1))


def _build_bass(Lsec, S, repeat=1):
    import concourse.bacc as bacc
    import concourse.tile as tile
    from concourse import mybir

    RL = max(Lsec)
    SLOT = _roundup(RL + 1, 16)  # 64B-aligned ring slots (gather base req)
    sec_off = [0]
    for L in Lsec:
        sec_off.append(sec_off[-1] + L)

    nc = bacc.Bacc("TRN2", target_bir_lowering=False, debug=False,
                   num_devices=NCORES, num_swdge_queues=2)
    table_d = nc.dram_tensor("table", [P, BANKA], mybir.dt.float32,
                             kind="ExternalInput").ap()
    idx_d = nc.dram_tensor("idx", [P, S // 16], mybir.dt.int16,
                           kind="ExternalInput").ap()
    dist_d = nc.dram_tensor("dist", [P, S], mybir.dt.bfloat16,
                            kind="ExternalInput").ap()
    eidx_d = nc.dram_tensor("eidx", [P, NSEC * EB // 16], mybir.dt.int16,
                            kind="ExternalInput").ap()
    out_d = nc.dram_tensor("out", [G * ATG, C], mybir.dt.float32,
                           kind="ExternalOutput").ap()
    RLW = _roundup(RL + 1, 16)  # padded slot width for tiles

    def act_recip(out, in_):
        eng = nc.scalar
        inputs = [eng.lower_ap(in_)]
        for arg in (0.0, 1.0, 0.0):
            inputs.append(mybir.ImmediateValue(dtype=mybir.dt.float32,
                                               value=arg))
        return eng.add_instruction(mybir.InstActivation(
            name=eng.bass.get_next_instruction_name(),
            func=mybir.ActivationFunctionType.Reciprocal,
            ins=inputs, outs=[eng.lower_ap(out)]))

    with tile.TileContext(nc) as tc:
        with tc.tile_pool(name="pers", bufs=1) as pers, \
             tc.tile_pool(name="ip", bufs=3) as ip, \
             tc.tile_pool(name="dp", bufs=2) as dp, \
             tc.tile_pool(name="gp", bufs=2) as gp, \
             tc.tile_pool(name="ep", bufs=2) as ep, \
             tc.tile_pool(name="fp", bufs=1) as fp:
            tab = pers.tile([P, BANKA], mybir.dt.float32)
            nc.sync.dma_start(tab[:], table_d[:])
            eix = pers.tile([P, NSEC * EB // 16], mybir.dt.int16)
            nc.sync.dma_start(eix[:], eidx_d[:])
            ringA = pers.tile([P, SLOT], mybir.dt.float32)
            ringB = pers.tile([P, SLOT], mybir.dt.float32)
            rings = [ringA, ringB]
            nc.gpsimd.memset(ringA[:], 0.0)
            nc.gpsimd.memset(ringB[:], 0.0)
            OUT = pers.tile([P, ATG], mybir.dt.float32)

            for _ in range(repeat):
                # pending E-gather emitted one section late so the gpsimd
                # queue runs [gather k+1][Egather k] without a bubble.
                pend = None

                def flush_pend():
                    nonlocal pend
                    if pend is None:
                        return
                    Ebp, segp, rgp, Lp, kp = pend
                    nc.gpsimd.ap_gather(
                        out_ap=Ebp[:, segp * EB:(segp + 1) * EB],
                        in_ap=rgp[:, 0:1 + Lp],
                        idxs_ap=eix[:, kp * (EB // 16):(kp + 1) * (EB // 16)],
                        channels=P, num_elems=1 + Lp, d=1, num_idxs=EB)
                    pend = None

                Eb = None
                prev_Eb = None
                for k in range(NSEC):
                    b, seg = k // SEGS, k % SEGS
                    rg = rings[k % 2]
                    L = Lsec[k]
                    if k > 0:
                        pg = rings[(k - 1) % 2]
                        nc.vector.tensor_copy(
                            rg[:, 0:1],
                            pg[:, Lsec[k - 1]:Lsec[k - 1] + 1])
                    u0 = sec_off[k]
                    dt_ = dp.tile([P, RLW], mybir.dt.bfloat16, tag="d")
                    nc.sync.dma_start(dt_[:, :L], dist_d[:, u0:u0 + L])
                    it_ = ip.tile([P, RLW // 16], mybir.dt.int16, tag="i")
                    nc.sync.dma_start(it_[:, :L // 16],
                                      idx_d[:, u0 // 16:(u0 + L) // 16])
                    rs = rg[:, 1:1 + L]
                    act_recip(rs, dt_[:, :L])
                    gt = gp.tile([P, RLW], mybir.dt.float32, tag="g")
                    nc.gpsimd.ap_gather(
                        out_ap=gt[:, :L], in_ap=tab[:],
                        idxs_ap=it_[:, :L // 16],
                        channels=P, num_elems=BANKA, d=1, num_idxs=L)
                    flush_pend()
                    nc.vector.scalar_tensor_tensor(
                        out=rs, in0=gt[:, :L], scalar=0.5, in1=rs,
                        op0=mybir.AluOpType.mult, op1=mybir.AluOpType.mult)
                    nc.vector.tensor_tensor_scan(
                        out=rs, data0=rs, data1=rs,
                        initial=rg[:, 0:1],
                        op0=mybir.AluOpType.add,
                        op1=mybir.AluOpType.bypass)
                    if seg == 0:
                        prev_Eb = Eb
                        Eb = ep.tile([P, SEGS * EB], mybir.dt.float32, tag="eb")
                    pend = (Eb, seg, rg, L, k)
                    if seg == 0 and b > 0:
                        _emit_bank_finish(nc, mybir, fp, OUT, prev_Eb, b - 1)
                flush_pend()
                _emit_bank_finish(nc, mybir, fp, OUT, Eb, BANKS - 1)
            o3 = out_d.rearrange("(g a) c -> g c a", g=G)
            for g in range(G):
                nc.sync.dma_start(o3[g:g + 1, :, :], OUT[16 * g:16 * g + 4, :])
    nc.finalize()
    return nc


def _emit_bank_finish(nc, mybir, fp, OUT, Eb, b):
    """Diff E entries, align bank rows 16g+4b+c onto 16g+c, accumulate."""
    D = fp.tile([P, SEGS * EB], mybir.dt.float32, tag="d2")
    nc.vector.tensor_tensor(
        out=D[:, :SEGS * EB - 1], in0=Eb[:, 1:],
        in1=Eb[:, :SEGS * EB - 1], op=mybir.AluOpType.subtract)
    Dv = D[:].rearrange("p (s e) -> p s e", e=EB)[:, :, 0:SEGA]
    SH = fp.tile([P, ATG], mybir.dt.float32, tag="sh")
    SHv = SH[:].rearrange("p (s e) -> p s e", e=SEGA)
    mask = (list(range(4 * b, 4 * b + 4)) + list(range(4, 16)) +
            list(range(16 + 4 * b, 16 + 4 * b + 4)) + list(range(20, 32)))
    nc.vector.stream_shuffle(SHv, Dv, mask)
    if b == 0:
        nc.vector.tensor_copy(OUT[:], SH[:])
    else:
        nc.vector.tensor_tensor(out=OUT[:], in0=OUT[:], in1=SH[:],
                                op=mybir.AluOpType.add)


class _Runner:
    def __init__(self, nc, n_cores):
        import jax
        from jax.sharding import Mesh, PartitionSpec
        try:
            from jax.experimental.shard_map import shard_map
        except Exception:
            from jax.sharding import shard_map
        from concourse import mybir
        from concourse.bass2jax import (_bass_exec_p, partition_id_tensor,
                                        install_neuronx_cc_hook)
        install_neuronx_cc_hook()
        self.jax = jax
        self.n_cores = n_cores
        pname = nc.partition_id_tensor.name if nc.partition_id_tensor else None
        in_names, out_names, out_avals, zero_outs = [], [], [], []
        for alloc in nc.m.functions[0].allocations:
            if not isinstance(alloc, mybir.MemoryLocationSet):
                continue
            name = alloc.memorylocations[0].name
            if alloc.kind == "ExternalInput":
                if name != pname:
                    in_names.append(name)
            elif alloc.kind == "ExternalOutput":
                shape = tuple(alloc.tensor_shape)
                dtype = mybir.dt.np(alloc.dtype)
                out_names.append(name)
                out_avals.append(jax.core.ShapedArray(shape, dtype))
                zero_outs.append(np.zeros(shape, dtype))
        self.in_names, self.out_names = in_names, out_names
        self.out_avals, self.zero_outs = out_avals, zero_outs
        n_params, n_outs = len(in_names), len(out_names)
        all_in = list(in_names) + list(out_names)
        if pname is not None:
            all_in.append(pname)

        def _body(*args):
            operands = list(args)
            if pname is not None:
                operands.append(partition_id_tensor())
            outs = _bass_exec_p.bind(
                *operands, out_avals=tuple(out_avals), in_names=tuple(all_in),
                out_names=tuple(out_names), lowering_input_output_aliases=(),
                sim_require_finite=False, sim_require_nnan=False, nc=nc)
            return tuple(outs)

        devices = jax.devices()[:n_cores]
        mesh = Mesh(np.asarray(devices), ("core",))
        in_specs = (PartitionSpec("core"),) * (n_params + n_outs)
        out_specs = (PartitionSpec("core"),) * n_outs
        self.fn = jax.jit(
            shard_map(_body, mesh=mesh, in_specs=in_specs,
                      out_specs=out_specs, check_rep=False),
            keep_unused=True)

    def run(self, in_maps):
        jax = self.jax
        concat_in = [
            np.concatenate([np.asarray(in_maps[c][n]) for c in range(self.n_cores)], axis=0)
            for n in self.in_names]
        concat_zeros = [
            np.zeros((self.n_cores * z.shape[0], *z.shape[1:]), z.dtype)
            for z in self.zero_outs]
        dargs = [jax.device_put(a) for a in concat_in + concat_zeros]
        outs = self.fn(*dargs)
        jax.block_until_ready(outs)
        res = []
        for c in range(self.n_cores):
            d = {}
            for i, n in enumerate(self.out_names):
                a = np.asarray(outs[i]).reshape(self.n_cores, *self.out_avals[i].shape)
                d[n] = a[c]
            res.append(d)
        return res


def kernel(charges, cell, positions, neighbor_indices, neighbor_distances):
    per_core, consts = _preprocess(neighbor_indices, neighbor_distances)
    key = (consts["Lsec"], consts["S"])
    if key not in _CACHE:
        nc = _build_bass(consts["Lsec"], consts["S"])
        _CACHE[key] = _Runner(nc, NCORES)
    runner = _CACHE[key]
    tab = _build_table(charges)
    in_maps = [{"table": tab, "idx": pc["idx"], "dist": pc["dist"],
                "eidx": pc["eidx"]} for pc in per_core]
    res = runner.run(in_maps)
    out = np.concatenate([res[k]["out"][:ATC] for k in range(NCORES)], axis=0)
    return np.ascontiguousarray(out, dtype=np.float32)
